# revision 2
# baseline (speedup 1.0000x reference)
"""Two-level VQ codebook assignment on 8 Trainium2 NeuronCores. v6.

Per 128-point chunk:
  1. PE: outer scores psO[p,64] = xt(stationary, host-packed feature-major
     hi/lo fp16) @ W1 (fp16, 64 moving cols).
  2. DVE: m = reduce-max(psO) [psum fp32]; Pool: H = is_equal(psO, m) fp16.
  3. PE: transpose H -> psum; Act: copy -> SBUF HsT.
  4. PE: gather ws[p,133] = HsT @ Wf (k-col + 4 blocks [db_j | 2(ic_j-ic_0)]).
  5. Pool: U = ws[:,1:] * xi (x2 fp16, bias slot); DVE/Pool: tree-sum -> ds[p,4].
  6. j* = argmax(0, ds): mf/ee/e2/jf; id = 5k + j.
Data-parallel over 8 cores; x host-packed per core:
  XT1 [ns,128,16,128] fp16: partition p=(w64+r): rows 0-15 x1hi, 16-31 x1lo,
  32-47 x1hi, 48-50 ones, rest 0; chunk c = 2*pair + w.
  XI [ns,128,CPS,33] fp16: [ones | x2h], point-major.
DMA split across SP (xt) and Act (xi, out) queues which run in parallel.
"""

import numpy as np

import bass_rust
import concourse.bass as bass
import concourse.mybir as mybir
import concourse.tile as tile
from concourse.bass_utils import run_bass_kernel_spmd
from concourse.masks import make_identity

N_TOTAL = 1_000_000
D1 = 16
D2 = 32
KO = 64
KI = 5
K1 = 51          # 16 hi + 16 lo + 16 hi + 3 bias rows
NW = 133         # k-col + 4 * (db | 2*dic[32])
N_CORES = 8
SUPER = 4096
CPS = SUPER // 128
B = 8            # chunks per batch
LAST_RESULTS = None

fp32 = mybir.dt.float32
fp16 = mybir.dt.float16
i32 = mybir.dt.int32


def _pad_to(n, m):
    return ((n + m - 1) // m) * m


def build_weights(outer_centers: np.ndarray, inner_centers: np.ndarray):
    oc = outer_centers.astype(np.float64)
    ic = inner_centers.astype(np.float64)

    W1 = np.zeros((K1, KO), dtype=np.float32)
    och = (2.0 * oc.T).astype(np.float16).astype(np.float32)
    ocl = (2.0 * oc.T - och).astype(np.float16).astype(np.float32)
    W1[0:16] = och
    W1[16:32] = och
    W1[32:48] = ocl
    bias = -np.sum(oc * oc, axis=1)
    b0 = bias.astype(np.float16).astype(np.float64)
    r = bias - b0
    b1 = r.astype(np.float16).astype(np.float64)
    b2 = (r - b1).astype(np.float16)
    W1[48] = b0
    W1[49] = b1
    W1[50] = b2

    bi = -np.sum(ic * ic, axis=2)            # [64, 5]
    Wf = np.zeros((KO, NW), dtype=np.float32)
    Wf[:, 0] = np.arange(KO)
    for j in range(1, KI):
        base = 1 + (j - 1) * 33
        Wf[:, base] = bi[:, j] - bi[:, 0]
        Wf[:, base + 1: base + 33] = 2.0 * (ic[:, j, :] - ic[:, 0, :])
    return W1.astype(np.float16), Wf.astype(np.float16)


def host_pack(xs: np.ndarray, n_super: int):
    """xs [n_pad, 48] fp32 -> XT1 [ns,128,16,128] fp16, XI [ns,128,CPS,33]."""
    f16 = np.float16
    x1 = xs[:, :D1].astype(np.float32)
    xh = x1.astype(f16)
    xl = (x1 - xh.astype(np.float32)).astype(f16)
    x2h = xs[:, D1:D1 + D2].astype(f16)

    # chunk c = 16*w + pair; [n, r] -> [s, w, pair, lane, r]
    def to5(a):
        return a.reshape(n_super, 2, CPS // 2, 128, a.shape[-1])

    xh5, xl5 = to5(xh), to5(xl)
    XT1 = np.zeros((n_super, 128, CPS // 2, 128), dtype=f16)
    for w in range(2):
        # [s, pair, lane, r] -> [s, r, pair, lane]
        hh = xh5[:, w].transpose(0, 3, 1, 2)
        ll = xl5[:, w].transpose(0, 3, 1, 2)
        XT1[:, 64 * w + 0: 64 * w + 16] = hh
        XT1[:, 64 * w + 16: 64 * w + 32] = ll
        XT1[:, 64 * w + 32: 64 * w + 48] = hh
        XT1[:, 64 * w + 48: 64 * w + 51] = 1.0

    XI = np.empty((n_super, 128, CPS, 33), dtype=f16)
    XI[..., 0] = 1.0
    # [n, d] -> [s, c, lane, d] -> [s, lane, c, d]
    XI[..., 1:] = x2h.reshape(n_super, CPS, 128, D2).transpose(0, 2, 1, 3)
    XM = np.concatenate([XT1.reshape(n_super, 128, -1),
                         XI.reshape(n_super, 128, -1)], axis=2)
    return XM


def split_waits(nc):
    for f in nc.m.functions:
        for b in f.blocks:
            out = []
            for inst in b.instructions:
                si = inst.sync_info
                if si is not None and len(si.on_wait) > 1:
                    waits = list(si.on_wait)
                    for i, w in enumerate(waits[:-1]):
                        nop = mybir.InstNoOp(name=f"{inst.name}-sw{i}", ins=[], outs=[])
                        nop.engine = inst.engine
                        nop.sync_info = bass_rust.SyncInfo(on_wait=[w], on_update=[])
                        out.append(nop)
                    inst.sync_info = bass_rust.SyncInfo(
                        on_wait=[waits[-1]], on_update=list(si.on_update)
                    )
                out.append(inst)
            b.instructions = out


def build_program(n_pad: int, for_hw: bool = True, stages: int = 99):
    assert n_pad % SUPER == 0
    n_super = n_pad // SUPER

    nc = bass.Bass()
    xm_ext = nc.declare_dram_parameter("xm", [n_super, 128, (CPS // 2) * 128 + CPS * 33],
                                       fp16, isOutput=False)
    w1_ext = nc.declare_dram_parameter("w1", [K1, KO], fp16, isOutput=False)
    wf_ext = nc.declare_dram_parameter("wf", [KO, NW], fp16, isOutput=False)
    out_ext = nc.declare_dram_parameter("out", [n_pad], i32, isOutput=True)

    with tile.TileContext(nc) as tc:
        with (
            tc.tile_pool(name="const", bufs=1) as constp,
            tc.tile_pool(name="xt", bufs=3) as xtp,
            tc.tile_pool(name="xi", bufs=3) as xip,
            tc.tile_pool(name="hh", bufs=4) as hhp,
            tc.tile_pool(name="hst", bufs=4) as hstp,
            tc.tile_pool(name="m8", bufs=6) as m8p,
            tc.tile_pool(name="uu", bufs=4) as uup,
            tc.tile_pool(name="tt", bufs=4) as ttp,
            tc.tile_pool(name="jj", bufs=4) as jjp,
            tc.tile_pool(name="ids", bufs=2) as idsp,
            tc.tile_pool(name="psO", bufs=2, space="PSUM") as psOp,
            tc.tile_pool(name="psH", bufs=2, space="PSUM") as psHp,
            tc.tile_pool(name="psW", bufs=2, space="PSUM") as psWp,
        ):
            identF = constp.tile([128, 128], fp16)
            make_identity(nc, identF[:])

            jio_i = constp.tile([128, 4], i32, tag="jio", name="jio")
            nc.gpsimd.iota(jio_i[:], pattern=[[-1, 4]], base=4, channel_multiplier=0)
            jiota = constp.tile([128, 4], fp16, tag="jiota", name="jiota")
            nc.gpsimd.tensor_copy(jiota[:], jio_i[:])

            w1_sb = constp.tile([128, KO], fp16, tag="w1", name="w1")
            nc.sync.dma_start(out=w1_sb[0:K1, :], in_=w1_ext[:])
            nc.sync.dma_start(out=w1_sb[64:64 + K1, :], in_=w1_ext[:])
            wf_sb = constp.tile([KO, NW], fp16, tag="wf", name="wf")
            nc.sync.dma_start(out=wf_sb[:], in_=wf_ext[:])

            def do_super(s):
                xm = xtp.tile([128, (CPS // 2) * 128 + CPS * 33], fp16)
                nc.sync.dma_start(out=xm[:], in_=xm_ext[s])
                xt = xm[:, 0:(CPS // 2) * 128].rearrange("p (a l) -> p a l", l=128)
                xiv = xm[:, (CPS // 2) * 128:].rearrange("p (c d) -> p c d", d=33)

                kacc = jjp.tile([128, CPS], fp16, tag="kacc")
                jsel = jjp.tile([128, CPS], fp16, tag="jsel")

                for g in range(CPS // B):
                    c0 = g * B
                    psO = psOp.tile([128, B, KO], fp32)
                    for b in range(B):
                        c = c0 + b
                        w, pair = c // (CPS // 2), c % (CPS // 2)
                        nc.tensor.matmul(
                            psO[:, b, :],
                            lhsT=xt[64 * w: 64 * w + K1, pair, :],
                            rhs=w1_sb[64 * w: 64 * w + K1, :],
                            start=True, stop=True,
                        )
                    if stages < 2:
                        continue
                    m8 = m8p.tile([128, B], fp32, tag="m8")
                    nc.vector.tensor_reduce(
                        out=m8[:], in_=psO[:], axis=mybir.AxisListType.X,
                        op=mybir.AluOpType.max,
                    )
                    hh = hhp.tile([128, B, KO], fp16, tag="hh")
                    nc.vector.tensor_tensor(
                        out=hh[:], in0=psO[:],
                        in1=m8[:].unsqueeze(-1).broadcast_to([128, B, KO]),
                        op=mybir.AluOpType.is_equal,
                    )
                    if stages < 3:
                        continue
                    pstH = psHp.tile([KO, B, 128], fp16)
                    for b in range(B):
                        nc.tensor.transpose(pstH[:, b, :], hh[:, b, :], identF[:])
                    hsT = hstp.tile([KO, B, 128], fp16)
                    nc.vector.tensor_copy(hsT[:, 0:2, :], pstH[:, 0:2, :])
                    nc.scalar.copy(hsT[:, 2:B, :], pstH[:, 2:B, :])

                    if stages < 4:
                        continue
                    wsb = uup.tile([128, B, NW], fp16, tag="wsb")
                    for hb in range(2):
                        h0 = hb * (B // 2)
                        ws = psWp.tile([128, B // 2, 256], fp32, tag="ws")
                        for b in range(B // 2):
                            nc.tensor.matmul(
                                ws[:, b, 0:NW], lhsT=hsT[:, h0 + b, :],
                                rhs=wf_sb[:], start=True, stop=True,
                            )
                        nc.scalar.copy(wsb[:, h0: h0 + B // 2, :], ws[:, :, 0:NW])

                    if stages < 5:
                        continue
                    uu = uup.tile([128, B, 4, 33], fp16, tag="uu")
                    nc.gpsimd.tensor_tensor(
                        out=uu[:],
                        in0=wsb[:, :, 1:NW].rearrange("p b (j d) -> p b j d", j=4),
                        in1=xiv[:, c0: c0 + B, :].unsqueeze(2)
                            .broadcast_to([128, B, 4, 33]),
                        op=mybir.AluOpType.mult,
                    )
                    # tree-sum over d=1..32 plus bias slot 0
                    t1 = ttp.tile([128, B, 4, 16], fp16, tag="t1")
                    nc.gpsimd.tensor_tensor(
                        out=t1[:], in0=uu[:, :, :, 1:17], in1=uu[:, :, :, 17:33],
                        op=mybir.AluOpType.add,
                    )
                    t2 = ttp.tile([128, B, 4, 8], fp16, tag="t2")
                    nc.gpsimd.tensor_tensor(
                        out=t2[:], in0=t1[:, :, :, 0:8], in1=t1[:, :, :, 8:16],
                        op=mybir.AluOpType.add,
                    )
                    t3 = ttp.tile([128, B, 4, 4], fp16, tag="t3")
                    nc.gpsimd.tensor_tensor(
                        out=t3[:], in0=t2[:, :, :, 0:4], in1=t2[:, :, :, 4:8],
                        op=mybir.AluOpType.add,
                    )
                    t4 = ttp.tile([128, B, 4, 2], fp16, tag="t4")
                    nc.gpsimd.tensor_tensor(
                        out=t4[:], in0=t3[:, :, :, 0:2], in1=t3[:, :, :, 2:4],
                        op=mybir.AluOpType.add,
                    )
                    t5 = ttp.tile([128, B, 4], fp16, tag="t5")
                    nc.gpsimd.tensor_tensor(
                        out=t5[:], in0=t4[:, :, :, 0], in1=t4[:, :, :, 1],
                        op=mybir.AluOpType.add,
                    )
                    ds = ttp.tile([128, B, 4], fp16, tag="ds")
                    nc.gpsimd.tensor_tensor(
                        out=ds[:], in0=t5[:], in1=uu[:, :, :, 0],
                        op=mybir.AluOpType.add,
                    )

                    if stages < 6:
                        continue
                    mf = m8p.tile([128, B], fp16, tag="mf")
                    nc.vector.tensor_reduce(
                        out=mf[:], in_=ds[:], axis=mybir.AxisListType.X,
                        op=mybir.AluOpType.max,
                    )
                    ee = ttp.tile([128, B, 4], fp16, tag="ee")
                    nc.vector.tensor_tensor(
                        out=ee[:], in0=ds[:],
                        in1=mf[:].unsqueeze(-1).broadcast_to([128, B, 4]),
                        op=mybir.AluOpType.is_equal,
                    )
                    e2 = ttp.tile([128, B, 4], fp16, tag="e2")
                    nc.gpsimd.tensor_tensor(
                        out=e2[:], in0=ee[:],
                        in1=jiota[:].unsqueeze(1).broadcast_to([128, B, 4]),
                        op=mybir.AluOpType.mult,
                    )
                    # m2 = max(ee * [4,3,2,1]) -> smallest tied j wins
                    m2 = m8p.tile([128, B], fp16, tag="m2")
                    nc.vector.tensor_reduce(
                        out=m2[:], in_=e2[:], axis=mybir.AxisListType.X,
                        op=mybir.AluOpType.max,
                    )
                    jpos = m8p.tile([128, B], fp16, tag="jpos")
                    nc.vector.tensor_scalar(
                        out=jpos[:], in0=mf[:], scalar1=0.0, scalar2=None,
                        op0=mybir.AluOpType.is_gt,
                    )
                    # id = 5*(k + jpos) - jpos*m2  (j = jpos*(5 - m2))
                    nc.gpsimd.tensor_tensor(
                        out=kacc[:, c0: c0 + B], in0=wsb[:, :, 0], in1=jpos[:],
                        op=mybir.AluOpType.add,
                    )
                    nc.gpsimd.tensor_tensor(
                        out=jsel[:, c0: c0 + B], in0=jpos[:], in1=m2[:],
                        op=mybir.AluOpType.mult,
                    )

                if stages < 6:
                    return
                ids_f = idsp.tile([128, CPS], fp16, tag="idsf")
                nc.vector.scalar_tensor_tensor(
                    out=ids_f[:], in0=kacc[:], scalar=5.0, in1=jsel[:],
                    op0=mybir.AluOpType.mult, op1=mybir.AluOpType.subtract,
                )
                ids_i = idsp.tile([128, CPS], i32, tag="idsi")
                nc.vector.tensor_copy(ids_i[:], ids_f[:])
                dst = out_ext[s * SUPER: (s + 1) * SUPER].rearrange(
                    "(a p) -> p a", p=128
                )
                nc.sync.dma_start(out=dst, in_=ids_i[:])

            for s in range(n_super):
                do_super(s)
    if for_hw:
        split_waits(nc)
    return nc


def kernel(x, outer_centers, inner_centers):
    global LAST_RESULTS
    x = np.ascontiguousarray(np.asarray(x, dtype=np.float32))
    W1, Wf = build_weights(np.asarray(outer_centers), np.asarray(inner_centers))

    n = x.shape[0]
    shard = (n + N_CORES - 1) // N_CORES
    n_pad = _pad_to(shard, SUPER)
    n_super = n_pad // SUPER

    nc = build_program(n_pad)

    in_maps = []
    for i in range(N_CORES):
        xs = x[i * shard: min((i + 1) * shard, n)]
        if xs.shape[0] < n_pad:
            xs = np.pad(xs, ((0, n_pad - xs.shape[0]), (0, 0)))
        XM = host_pack(xs, n_super)
        in_maps.append({"xm": XM, "w1": W1, "wf": Wf})

    res = run_bass_kernel_spmd(nc, in_maps, list(range(N_CORES)), trace=False)
    LAST_RESULTS = res
    outs = []
    for i in range(N_CORES):
        lo = i * shard
        hi = min((i + 1) * shard, n)
        outs.append(res.results[i]["out"][: hi - lo])
    return np.concatenate(outs).astype(np.int32)


# revision 4
# speedup vs baseline: 1.0695x; 1.0695x over previous
"""Two-level VQ codebook assignment on 8 Trainium2 NeuronCores. v6.

Per 128-point chunk:
  1. PE: outer scores psO[p,64] = xt(stationary, host-packed feature-major
     hi/lo fp16) @ W1 (fp16, 64 moving cols).
  2. DVE: m = reduce-max(psO) [psum fp32]; Pool: H = is_equal(psO, m) fp16.
  3. PE: transpose H -> psum; Act: copy -> SBUF HsT.
  4. PE: gather ws[p,133] = HsT @ Wf (k-col + 4 blocks [db_j | 2(ic_j-ic_0)]).
  5. Pool: U = ws[:,1:] * xi (x2 fp16, bias slot); DVE/Pool: tree-sum -> ds[p,4].
  6. j* = argmax(0, ds): mf/ee/e2/jf; id = 5k + j.
Data-parallel over 8 cores; x host-packed per core:
  XT1 [ns,128,16,128] fp16: partition p=(w64+r): rows 0-15 x1hi, 16-31 x1lo,
  32-47 x1hi, 48-50 ones, rest 0; chunk c = 2*pair + w.
  XI [ns,128,CPS,33] fp16: [ones | x2h], point-major.
DMA split across SP (xt) and Act (xi, out) queues which run in parallel.
"""

import numpy as np

import bass_rust
import concourse.bass as bass
import concourse.mybir as mybir
import concourse.tile as tile
from concourse.bass_utils import run_bass_kernel_spmd
from concourse.masks import make_identity

N_TOTAL = 1_000_000
D1 = 16
D2 = 32
KO = 64
KI = 5
K1 = 51          # 16 hi + 16 lo + 16 hi + 3 bias rows
NW = 133         # k-col + 4 * (db | 2*dic[32])
N_CORES = 8
SUPER = 4096
CPS = SUPER // 128
B = 8            # chunks per batch
LAST_RESULTS = None

fp32 = mybir.dt.float32
fp16 = mybir.dt.float16
i32 = mybir.dt.int32


def _pad_to(n, m):
    return ((n + m - 1) // m) * m


def build_weights(outer_centers: np.ndarray, inner_centers: np.ndarray):
    oc = outer_centers.astype(np.float64)
    ic = inner_centers.astype(np.float64)

    W1 = np.zeros((K1, KO), dtype=np.float32)
    och = (2.0 * oc.T).astype(np.float16).astype(np.float32)
    ocl = (2.0 * oc.T - och).astype(np.float16).astype(np.float32)
    W1[0:16] = och
    W1[16:32] = och
    W1[32:48] = ocl
    bias = -np.sum(oc * oc, axis=1)
    b0 = bias.astype(np.float16).astype(np.float64)
    r = bias - b0
    b1 = r.astype(np.float16).astype(np.float64)
    b2 = (r - b1).astype(np.float16)
    W1[48] = b0
    W1[49] = b1
    W1[50] = b2

    bi = -np.sum(ic * ic, axis=2)            # [64, 5]
    Wf = np.zeros((KO, NW), dtype=np.float32)
    Wf[:, 0] = np.arange(KO)
    for j in range(1, KI):
        base = 1 + (j - 1) * 33
        Wf[:, base] = bi[:, j] - bi[:, 0]
        Wf[:, base + 1: base + 33] = 2.0 * (ic[:, j, :] - ic[:, 0, :])
    return W1.astype(np.float16), Wf.astype(np.float16)


def host_pack(xs: np.ndarray, n_super: int):
    """xs [n_pad, 48] fp32 -> XT1 [ns,128,16,128] fp16, XI [ns,128,CPS,33]."""
    f16 = np.float16
    x1 = xs[:, :D1].astype(np.float32)
    xh = x1.astype(f16)
    xl = (x1 - xh.astype(np.float32)).astype(f16)
    x2h = xs[:, D1:D1 + D2].astype(f16)

    # chunk c = 16*w + pair; [n, r] -> [s, w, pair, lane, r]
    def to5(a):
        return a.reshape(n_super, 2, CPS // 2, 128, a.shape[-1])

    xh5, xl5 = to5(xh), to5(xl)
    XT1 = np.zeros((n_super, 128, CPS // 2, 128), dtype=f16)
    for w in range(2):
        # [s, pair, lane, r] -> [s, r, pair, lane]
        hh = xh5[:, w].transpose(0, 3, 1, 2)
        ll = xl5[:, w].transpose(0, 3, 1, 2)
        XT1[:, 64 * w + 0: 64 * w + 16] = hh
        XT1[:, 64 * w + 16: 64 * w + 32] = ll
        XT1[:, 64 * w + 32: 64 * w + 48] = hh
        XT1[:, 64 * w + 48: 64 * w + 51] = 1.0

    XI = np.empty((n_super, 128, CPS, 33), dtype=f16)
    XI[..., 0] = 1.0
    # [n, d] -> [s, c, lane, d] -> [s, lane, c, d]
    XI[..., 1:] = x2h.reshape(n_super, CPS, 128, D2).transpose(0, 2, 1, 3)
    XM = np.concatenate([XT1.reshape(n_super, 128, -1),
                         XI.reshape(n_super, 128, -1)], axis=2)
    return XM


def split_waits(nc):
    for f in nc.m.functions:
        for b in f.blocks:
            out = []
            for inst in b.instructions:
                si = inst.sync_info
                if si is not None and len(si.on_wait) > 1:
                    waits = list(si.on_wait)
                    for i, w in enumerate(waits[:-1]):
                        nop = mybir.InstNoOp(name=f"{inst.name}-sw{i}", ins=[], outs=[])
                        nop.engine = inst.engine
                        nop.sync_info = bass_rust.SyncInfo(on_wait=[w], on_update=[])
                        out.append(nop)
                    inst.sync_info = bass_rust.SyncInfo(
                        on_wait=[waits[-1]], on_update=list(si.on_update)
                    )
                out.append(inst)
            b.instructions = out


def build_program(n_pad: int, for_hw: bool = True, stages: int = 99):
    assert n_pad % SUPER == 0
    n_super = n_pad // SUPER

    nc = bass.Bass()
    xm_ext = nc.declare_dram_parameter("xm", [n_super, 128, (CPS // 2) * 128 + CPS * 33],
                                       fp16, isOutput=False)
    w1_ext = nc.declare_dram_parameter("w1", [K1, KO], fp16, isOutput=False)
    wf_ext = nc.declare_dram_parameter("wf", [KO, NW], fp16, isOutput=False)
    out_ext = nc.declare_dram_parameter("out", [n_pad], i32, isOutput=True)

    with tile.TileContext(nc) as tc:
        with (
            tc.tile_pool(name="const", bufs=1) as constp,
            tc.tile_pool(name="xt", bufs=4) as xtp,
            tc.tile_pool(name="xi", bufs=3) as xip,
            tc.tile_pool(name="hh", bufs=6) as hhp,
            tc.tile_pool(name="hst", bufs=6) as hstp,
            tc.tile_pool(name="m8", bufs=6) as m8p,
            tc.tile_pool(name="uu", bufs=6) as uup,
            tc.tile_pool(name="tt", bufs=6) as ttp,
            tc.tile_pool(name="jj", bufs=4) as jjp,
            tc.tile_pool(name="ids", bufs=2) as idsp,
            tc.tile_pool(name="psO", bufs=2, space="PSUM") as psOp,
            tc.tile_pool(name="psH", bufs=2, space="PSUM") as psHp,
            tc.tile_pool(name="psW", bufs=2, space="PSUM") as psWp,
        ):
            identF = constp.tile([128, 128], fp16)
            make_identity(nc, identF[:])

            jio_i = constp.tile([128, 4], i32, tag="jio", name="jio")
            nc.gpsimd.iota(jio_i[:], pattern=[[-1, 4]], base=4, channel_multiplier=0)
            jiota = constp.tile([128, 4], fp16, tag="jiota", name="jiota")
            nc.gpsimd.tensor_copy(jiota[:], jio_i[:])

            w1_sb = constp.tile([128, KO], fp16, tag="w1", name="w1")
            nc.sync.dma_start(out=w1_sb[0:K1, :], in_=w1_ext[:])
            nc.sync.dma_start(out=w1_sb[64:64 + K1, :], in_=w1_ext[:])
            wf_sb = constp.tile([KO, NW], fp16, tag="wf", name="wf")
            nc.sync.dma_start(out=wf_sb[:], in_=wf_ext[:])

            state = {}

            def do_super(s):
                xm = xtp.tile([128, (CPS // 2) * 128 + CPS * 33], fp16)
                nc.sync.dma_start(out=xm[:], in_=xm_ext[s])
                xt = xm[:, 0:(CPS // 2) * 128].rearrange("p (a l) -> p a l", l=128)
                xiv = xm[:, (CPS // 2) * 128:].rearrange("p (c d) -> p c d", d=33)

                kacc = jjp.tile([128, CPS], fp16, tag="kacc")
                jsel = jjp.tile([128, CPS], fp16, tag="jsel")
                dss = jjp.tile([128, CPS, 4], fp16, tag="dss")

                for g in range(CPS // B):
                    c0 = g * B
                    psO = psOp.tile([128, B, KO], fp32)
                    for b in range(B):
                        c = c0 + b
                        w, pair = c // (CPS // 2), c % (CPS // 2)
                        nc.tensor.matmul(
                            psO[:, b, :],
                            lhsT=xt[64 * w: 64 * w + K1, pair, :],
                            rhs=w1_sb[64 * w: 64 * w + K1, :],
                            start=True, stop=True,
                        )
                    if stages < 2:
                        continue
                    m8 = m8p.tile([128, B], fp32, tag="m8")
                    nc.vector.tensor_reduce(
                        out=m8[:], in_=psO[:], axis=mybir.AxisListType.X,
                        op=mybir.AluOpType.max,
                    )
                    hh = hhp.tile([128, B, KO], fp16, tag="hh")
                    nc.vector.tensor_tensor(
                        out=hh[:], in0=psO[:],
                        in1=m8[:].unsqueeze(-1).broadcast_to([128, B, KO]),
                        op=mybir.AluOpType.is_equal,
                    )
                    if stages < 3:
                        continue
                    pstH = psHp.tile([KO, B, 128], fp16)
                    for b in range(B):
                        nc.tensor.transpose(pstH[:, b, :], hh[:, b, :], identF[:])
                    hsT = hstp.tile([KO, B, 128], fp16)
                    nc.vector.tensor_copy(hsT[:, 0:4, :], pstH[:, 0:4, :])
                    nc.scalar.copy(hsT[:, 4:B, :], pstH[:, 4:B, :])

                    if stages < 4:
                        continue
                    wsb = uup.tile([128, B, NW], fp16, tag="wsb")
                    for hb in range(2):
                        h0 = hb * (B // 2)
                        ws = psWp.tile([128, B // 2, 256], fp32, tag="ws")
                        for b in range(B // 2):
                            nc.tensor.matmul(
                                ws[:, b, 0:NW], lhsT=hsT[:, h0 + b, :],
                                rhs=wf_sb[:], start=True, stop=True,
                            )
                        nc.scalar.copy(wsb[:, h0: h0 + B // 2, :], ws[:, :, 0:NW])

                    if stages < 5:
                        continue
                    uu = uup.tile([128, B, 4, 33], fp16, tag="uu")
                    nc.gpsimd.tensor_tensor(
                        out=uu[:],
                        in0=wsb[:, :, 1:NW].rearrange("p b (j d) -> p b j d", j=4),
                        in1=xiv[:, c0: c0 + B, :].unsqueeze(2)
                            .broadcast_to([128, B, 4, 33]),
                        op=mybir.AluOpType.mult,
                    )
                    # tree-sum over d=1..32 plus bias slot 0
                    t1 = ttp.tile([128, B, 4, 16], fp16, tag="t1")
                    nc.gpsimd.tensor_tensor(
                        out=t1[:], in0=uu[:, :, :, 1:17], in1=uu[:, :, :, 17:33],
                        op=mybir.AluOpType.add,
                    )
                    t2 = ttp.tile([128, B, 4, 8], fp16, tag="t2")
                    nc.gpsimd.tensor_tensor(
                        out=t2[:], in0=t1[:, :, :, 0:8], in1=t1[:, :, :, 8:16],
                        op=mybir.AluOpType.add,
                    )
                    t3 = ttp.tile([128, B, 4, 4], fp16, tag="t3")
                    nc.gpsimd.tensor_tensor(
                        out=t3[:], in0=t2[:, :, :, 0:4], in1=t2[:, :, :, 4:8],
                        op=mybir.AluOpType.add,
                    )
                    t4 = ttp.tile([128, B, 4, 2], fp16, tag="t4")
                    nc.gpsimd.tensor_tensor(
                        out=t4[:], in0=t3[:, :, :, 0:2], in1=t3[:, :, :, 2:4],
                        op=mybir.AluOpType.add,
                    )
                    t5 = ttp.tile([128, B, 4], fp16, tag="t5")
                    nc.gpsimd.tensor_tensor(
                        out=t5[:], in0=t4[:, :, :, 0], in1=t4[:, :, :, 1],
                        op=mybir.AluOpType.add,
                    )
                    nc.gpsimd.tensor_tensor(
                        out=dss[:, c0: c0 + B, :], in0=t5[:], in1=uu[:, :, :, 0],
                        op=mybir.AluOpType.add,
                    )
                    if stages < 6:
                        continue
                    nc.gpsimd.tensor_copy(kacc[:, c0: c0 + B], wsb[:, :, 0])

                # per-super j* extraction on [128, CPS, 4]
                if stages < 6:
                    return
                mf = m8p.tile([128, CPS], fp16, tag="mf")
                nc.vector.tensor_reduce(
                    out=mf[:], in_=dss[:], axis=mybir.AxisListType.X,
                    op=mybir.AluOpType.max,
                )
                ee = ttp.tile([128, CPS, 4], fp16, tag="ee")
                nc.vector.tensor_tensor(
                    out=ee[:], in0=dss[:],
                    in1=mf[:].unsqueeze(-1).broadcast_to([128, CPS, 4]),
                    op=mybir.AluOpType.is_equal,
                )
                e2 = ttp.tile([128, CPS, 4], fp16, tag="e2")
                nc.gpsimd.tensor_tensor(
                    out=e2[:], in0=ee[:],
                    in1=jiota[:].unsqueeze(1).broadcast_to([128, CPS, 4]),
                    op=mybir.AluOpType.mult,
                )
                # m2 = max(ee * [4,3,2,1]) -> smallest tied j wins
                m2 = m8p.tile([128, CPS], fp16, tag="m2")
                nc.vector.tensor_reduce(
                    out=m2[:], in_=e2[:], axis=mybir.AxisListType.X,
                    op=mybir.AluOpType.max,
                )
                jpos = m8p.tile([128, CPS], fp16, tag="jpos")
                nc.vector.tensor_scalar(
                    out=jpos[:], in0=mf[:], scalar1=0.0, scalar2=None,
                    op0=mybir.AluOpType.is_gt,
                )
                # id = 5*(k + jpos) - jpos*m2  (j = jpos*(5 - m2))
                nc.gpsimd.tensor_tensor(
                    out=kacc[:], in0=kacc[:], in1=jpos[:],
                    op=mybir.AluOpType.add,
                )
                nc.gpsimd.tensor_tensor(
                    out=jsel[:], in0=jpos[:], in1=m2[:],
                    op=mybir.AluOpType.mult,
                )

                ids_f = idsp.tile([128, CPS], fp16, tag="idsf")
                nc.vector.scalar_tensor_tensor(
                    out=ids_f[:], in0=kacc[:], scalar=5.0, in1=jsel[:],
                    op0=mybir.AluOpType.mult, op1=mybir.AluOpType.subtract,
                )
                if s % 2 == 0:
                    state["ids_i"] = idsp.tile([128, 2 * CPS], i32, tag="idsi", name="ids_i")
                ids_i = state["ids_i"]
                half = s % 2
                nc.vector.tensor_copy(ids_i[:, half * CPS:(half + 1) * CPS], ids_f[:])
                if half == 1 or s == n_super - 1:
                    s0 = s - half
                    dst = out_ext[s0 * SUPER: (s + 1) * SUPER].rearrange(
                        "(a p) -> p a", p=128
                    )
                    nc.sync.dma_start(out=dst, in_=ids_i[:, 0:(half + 1) * CPS])

            for s in range(n_super):
                do_super(s)
    if for_hw:
        split_waits(nc)
    return nc


def kernel(x, outer_centers, inner_centers):
    global LAST_RESULTS
    x = np.ascontiguousarray(np.asarray(x, dtype=np.float32))
    W1, Wf = build_weights(np.asarray(outer_centers), np.asarray(inner_centers))

    n = x.shape[0]
    shard = (n + N_CORES - 1) // N_CORES
    n_pad = _pad_to(shard, SUPER)
    n_super = n_pad // SUPER

    nc = build_program(n_pad)

    in_maps = []
    for i in range(N_CORES):
        xs = x[i * shard: min((i + 1) * shard, n)]
        if xs.shape[0] < n_pad:
            xs = np.pad(xs, ((0, n_pad - xs.shape[0]), (0, 0)))
        XM = host_pack(xs, n_super)
        in_maps.append({"xm": XM, "w1": W1, "wf": Wf})

    res = run_bass_kernel_spmd(nc, in_maps, list(range(N_CORES)), trace=False)
    LAST_RESULTS = res
    outs = []
    for i in range(N_CORES):
        lo = i * shard
        hi = min((i + 1) * shard, n)
        outs.append(res.results[i]["out"][: hi - lo])
    return np.concatenate(outs).astype(np.int32)
